# revision 5
# baseline (speedup 1.0000x reference)
"""Multi-head attention + output projection, sharded over 8 TRN2 NeuronCores.

Problem: Q,K,V [4,1024,1024] f32; 16 heads x 64 dim; softmax(QK^T/sqrt(1024))V,
concat heads, out @ W_H.T + b_H.

Sharding: 8 cores = 4 batch x 2 query-halves. Each core computes full attention
(all 16 heads, all 1024 keys) for its 512 queries plus the output projection for
those rows. Output rows are disjoint -> no collectives.

Per-core kernel (all matmuls bf16, fp32 accumulation):
  scoresT[k,q] = KhT.T @ QhT  (contract d=64; Q pre-scaled by 1/sqrt(D) on host)
  expT = exp(scoresT)  (ACT, psum->sbuf, bf16 out; no max-subtraction needed:
                        |scores/32| <= ~1.5 for N(0,1) inputs)
  outT_aug[65,q] = sum_k V_aug[k,65].T @ expT[k,q]  (V augmented with a ones
                        column -> row 64 = softmax denominator)
  normalize via reciprocal + PE outer-product broadcast, then
  final[q,n] = outT_norm.T @ W_H.T + b_H  (contract over 1024 = 8 chunks)
"""
import sys
import os

sys.path.insert(0, "/opt/trn_rl_repo")

import numpy as np
import ml_dtypes

B, L, D, H, HD = 4, 1024, 1024, 16, 64
NCORES = 8
QBLK = L // 2  # 512 queries per core
SCALE = 1.0 / np.sqrt(np.float32(D))

_STATE = {}


def _build_nc(niter=1):
    import concourse.bass as bass
    import concourse.tile as tile
    from concourse import bacc, mybir
    from contextlib import ExitStack

    F32 = mybir.dt.float32
    F32R = mybir.dt.float32r
    BF16 = mybir.dt.bfloat16
    Exp = mybir.ActivationFunctionType.Exp

    nc = bacc.Bacc("TRN2", target_bir_lowering=False, debug=False)
    qt = nc.dram_tensor("qt", [128, 8, QBLK], F32R, kind="ExternalInput")
    kt = nc.dram_tensor("kt", [128, 8, L], F32R, kind="ExternalInput")
    vv = nc.dram_tensor("vv", [128, H, 8, HD + 1], F32R, kind="ExternalInput")
    wht = nc.dram_tensor("wht", [128, 8, D], F32R, kind="ExternalInput")
    bias = nc.dram_tensor("bias", [128, D], F32, kind="ExternalInput")
    out = nc.dram_tensor("out", [QBLK, D], F32, kind="ExternalOutput")

    with tile.TileContext(nc) as tc, ExitStack() as ctx:
        singles = ctx.enter_context(tc.tile_pool(name="singles", bufs=1))
        exp_pool = ctx.enter_context(tc.tile_pool(name="exp", bufs=2))
        norm_pool = ctx.enter_context(tc.tile_pool(name="norm", bufs=3))
        final_pool = ctx.enter_context(tc.tile_pool(name="final", bufs=2))
        scores_ps = ctx.enter_context(tc.tile_pool(name="scps", bufs=2, space="PSUM"))
        ov_ps = ctx.enter_context(tc.tile_pool(name="ovps", bufs=1, space="PSUM"))
        bc_ps = ctx.enter_context(tc.tile_pool(name="bcps", bufs=1, space="PSUM"))
        proj_ps = ctx.enter_context(tc.tile_pool(name="prps", bufs=2, space="PSUM"))

        def body(_=None):
            # warm the exp table while DMAs run
            warm_in = singles.tile([1, 8], F32, tag="warm_in")
            warm_out = singles.tile([1, 8], F32, tag="warm_out")
            nc.vector.memset(warm_in, 0.0)
            nc.scalar.activation(out=warm_out, in_=warm_in, func=Exp)

            sb_ones_f = singles.tile([1, HD], F32, tag="ones_f")
            nc.vector.memset(sb_ones_f, 1.0)
            sb_ones = singles.tile([1, HD], F32R, tag="ones")
            nc.vector.tensor_copy(out=sb_ones, in_=sb_ones_f)
            sb_bias = singles.tile([128, D], F32, tag="bias")
            nc.sync.dma_start(sb_bias, bias.ap())

            sb_qt = singles.tile([128, 8, QBLK], F32R, tag="qt")
            sb_kt = singles.tile([128, 8, L], F32R, tag="kt")
            sb_v = singles.tile([128, H, 8, HD + 1], F32R, tag="v")
            for j in range(8):
                nc.sync.dma_start(sb_qt[:, j], qt.ap()[:, j])
                nc.sync.dma_start(sb_kt[:, j], kt.ap()[:, j])
                nc.sync.dma_start(sb_v[:, 2 * j], vv.ap()[:, 2 * j])
                nc.sync.dma_start(sb_v[:, 2 * j + 1], vv.ap()[:, 2 * j + 1])
            sb_wht = singles.tile([128, 8, D], F32R, tag="wht")
            for cc in range(8):
                nc.sync.dma_start(sb_wht[:, cc], wht.ap()[:, cc])

            # normalized concatenated attention output, transposed: [hd, q]
            outT = singles.tile([128, 8, QBLK], F32R, tag="outT")

            for h in range(H):
                j, par = divmod(h, 2)
                poff = par * HD
                expT = exp_pool.tile([128, 8, QBLK], F32R, tag="expT")
                ov = ov_ps.tile([HD + 1, QBLK], F32, tag="ov")

                def scores_subround(r):
                    S = scores_ps.tile([128, 2, QBLK], F32, tag="S")
                    for i in range(2):
                        c = 2 * r + i
                        nc.tensor.matmul(
                            S[:, i, :],
                            lhsT=sb_kt[poff:poff + HD, j, c * 128:(c + 1) * 128],
                            rhs=sb_qt[poff:poff + HD, j, :],
                            start=True,
                            stop=True,
                        )
                    nc.scalar.activation(
                        out=expT[:, 2 * r:2 * r + 2, :], in_=S[:, :, :], func=Exp
                    )

                def attnv(r):
                    for i in range(2):
                        c = 2 * r + i
                        nc.tensor.matmul(
                            ov[:, :],
                            lhsT=sb_v[:, h, c, :],
                            rhs=expT[:, c, :],
                            start=(c == 0),
                            stop=(c == 7),
                        )

                scores_subround(0)
                scores_subround(1)
                attnv(0)
                scores_subround(2)
                attnv(1)
                scores_subround(3)
                attnv(2)
                attnv(3)

                recip = norm_pool.tile([1, QBLK], F32R, tag="recip")
                with nc.allow_low_precision(reason="f32r recip feeds f32r matmul"):
                    nc.vector.reciprocal(out=recip, in_=ov[HD:HD + 1, :])
                bc = bc_ps.tile([HD, QBLK], F32, tag="bc")
                # fp32 outer product: ones[1,64].T @ recip[1,512] -> [64,512]
                nc.tensor.matmul(bc, lhsT=sb_ones, rhs=recip, start=True, stop=True)
                bc_sb = norm_pool.tile([HD, QBLK], F32, tag="bc_sb")
                nc.vector.tensor_copy(out=bc_sb, in_=bc)
                if par == 0:
                    nc.vector.tensor_mul(
                        out=outT[0:HD, j, :], in0=ov[0:HD, :], in1=bc_sb
                    )
                else:
                    tmp = norm_pool.tile([HD, QBLK], F32R, tag="tmp")
                    nc.vector.tensor_mul(out=tmp, in0=ov[0:HD, :], in1=bc_sb)
                    nc.sync.dma_start(outT[HD:128, j, :], tmp)

            # output projection: final[q, n] = outT.T @ WHT + bias
            for m in range(QBLK // 128):
                for jn in range(D // 512):
                    P = proj_ps.tile([128, 512], F32, tag="P")
                    for cc in range(8):
                        nc.tensor.matmul(
                            P,
                            lhsT=outT[:, cc, m * 128:(m + 1) * 128],
                            rhs=sb_wht[:, cc, jn * 512:(jn + 1) * 512],
                            start=(cc == 0),
                            stop=(cc == 7),
                        )
                    F = final_pool.tile([128, 512], F32, tag="F")
                    nc.vector.tensor_add(
                        out=F, in0=P, in1=sb_bias[:, jn * 512:(jn + 1) * 512]
                    )
                    nc.sync.dma_start(
                        out.ap()[m * 128:(m + 1) * 128, jn * 512:(jn + 1) * 512], F
                    )

        if niter == 1:
            body()
        else:
            with tc.For_i(
                0, niter, 1,
                hint_engines=(
                    mybir.EngineType.PE,
                    mybir.EngineType.Activation,
                    mybir.EngineType.DVE,
                    mybir.EngineType.SP,
                ),
            ) as _i:
                body(_i)

    nc.compile()
    return nc


def _host_shard(Q, K, V, W_H, b_H):
    """Build the 8 per-core input dicts (all host-side numpy)."""
    Qs = (np.asarray(Q, np.float32) * SCALE)
    K = np.asarray(K, np.float32)
    V = np.asarray(V, np.float32)
    W_H = np.asarray(W_H, np.float32)
    b_H = np.asarray(b_H, np.float32)

    # [hd, n] chunked: [128, 8, D]
    wht = np.ascontiguousarray(
        W_H.T.reshape(8, 128, D).transpose(1, 0, 2)
    )
    bias = np.ascontiguousarray(np.broadcast_to(b_H, (128, D))).astype(np.float32)

    in_maps = []
    for c in range(NCORES):
        b, half = divmod(c, 2)
        qlo = half * QBLK
        # [q, j, par, d] -> [par, d, j, q] -> [128, 8, QBLK]
        qt = np.ascontiguousarray(
            Qs[b, qlo:qlo + QBLK].reshape(QBLK, 8, 2, HD).transpose(2, 3, 1, 0)
        ).reshape(128, 8, QBLK)
        kt = np.ascontiguousarray(
            K[b].reshape(L, 8, 2, HD).transpose(2, 3, 1, 0)
        ).reshape(128, 8, L)
        # V_aug [k, h, 65] -> [c, p, h, e] -> [p, h, c, e]
        va = np.concatenate(
            [V[b].reshape(L, H, HD), np.ones((L, H, 1), np.float32)], axis=2
        )
        vv = np.ascontiguousarray(
            va.reshape(8, 128, H, HD + 1).transpose(1, 2, 0, 3)
        )
        in_maps.append({"qt": qt, "kt": kt, "vv": vv, "wht": wht, "bias": bias})
    return in_maps


def _get_runner(niter=1):
    """Build (once) and cache a jitted 8-core runner for the kernel."""
    key = ("runner", niter)
    if key in _STATE:
        return _STATE[key]

    import jax
    import jax.numpy as jnp
    from jax.sharding import Mesh, PartitionSpec
    from jax.experimental.shard_map import shard_map
    from concourse import bass2jax, mybir

    nc = _build_nc(niter)
    bass2jax.install_neuronx_cc_hook()

    partition_name = (
        nc.partition_id_tensor.name if nc.partition_id_tensor else None
    )
    in_names, out_names, out_avals, zero_shapes = [], [], [], []
    for alloc in nc.m.functions[0].allocations:
        if not isinstance(alloc, mybir.MemoryLocationSet):
            continue
        name = alloc.memorylocations[0].name
        if alloc.kind == "ExternalInput":
            if name != partition_name:
                in_names.append(name)
        elif alloc.kind == "ExternalOutput":
            out_names.append(name)
            shape = tuple(alloc.tensor_shape)
            dtype = mybir.dt.np(alloc.dtype)
            out_avals.append(jax.core.ShapedArray(shape, dtype))
            zero_shapes.append((shape, dtype))
    n_params = len(in_names)
    n_outs = len(out_avals)
    all_names = list(in_names) + list(out_names)
    if partition_name is not None:
        all_names.append(partition_name)
    donate = tuple(range(n_params, n_params + n_outs))

    def _body(*args):
        operands = list(args)
        if partition_name is not None:
            operands.append(bass2jax.partition_id_tensor())
        outs = bass2jax._bass_exec_p.bind(
            *operands,
            out_avals=tuple(out_avals),
            in_names=tuple(all_names),
            out_names=tuple(out_names),
            lowering_input_output_aliases=(),
            sim_require_finite=True,
            sim_require_nnan=True,
            nc=nc,
        )
        return tuple(outs)

    devices = jax.devices()[:NCORES]
    mesh = Mesh(np.asarray(devices), ("core",))
    in_specs = (PartitionSpec("core"),) * (n_params + n_outs)
    out_specs = (PartitionSpec("core"),) * n_outs
    sharded = jax.jit(
        shard_map(
            _body, mesh=mesh, in_specs=in_specs, out_specs=out_specs,
            check_rep=False,
        ),
        donate_argnums=donate,
        keep_unused=True,
    )

    def run(in_maps, device_inputs=None):
        if device_inputs is None:
            device_inputs = put_inputs(in_maps)
        zeros = [
            np.zeros((NCORES * s[0], *s[1:]), d) for s, d in zero_shapes
        ]
        out_arrs = sharded(*device_inputs, *zeros)
        results = []
        for c in range(NCORES):
            results.append({
                name: np.asarray(out_arrs[i]).reshape(
                    NCORES, *out_avals[i].shape
                )[c]
                for i, name in enumerate(out_names)
            })
        return results

    def put_inputs(in_maps):
        return [
            np.concatenate([np.asarray(in_maps[c][nm]) for c in range(NCORES)],
                           axis=0)
            for nm in in_names
        ]

    runner = {"run": run, "put_inputs": put_inputs, "sharded": sharded,
              "in_names": in_names, "out_names": out_names,
              "zero_shapes": zero_shapes, "nc": nc}
    _STATE[key] = runner
    return runner


def kernel(Q=None, K=None, V=None, W_H=None, b_H=None, mask=None, **kw):
    in_maps = _host_shard(Q, K, V, W_H, b_H)
    runner = _get_runner(niter=1)
    results = runner["run"](in_maps)
    out = np.empty((B, L, D), np.float32)
    for c in range(NCORES):
        b, half = divmod(c, 2)
        out[b, half * QBLK:(half + 1) * QBLK, :] = results[c]["out"]
    return out


# revision 7
# speedup vs baseline: 7.5420x; 7.5420x over previous
"""Multi-head attention + output projection, sharded over 8 TRN2 NeuronCores.

Problem: Q,K,V [4,1024,1024] f32; 16 heads x 64 dim; softmax(QK^T/sqrt(1024))V,
concat heads, out @ W_H.T + b_H.

Sharding: 8 cores = 4 batch x 2 query-halves. Each core computes full attention
(all 16 heads, all 1024 keys) for its 512 queries plus the output projection for
those rows. Output rows are disjoint -> no collectives.

Per-core kernel (all matmuls float32r: full PE rate at N>=512, ~1.6e-4 matmul
precision; fp32 PSUM accumulation):
  scoresT[k,q] = KhT.T @ QhT  (contract d=64; Q pre-scaled by 1/sqrt(D) on host)
  expT = exp(scoresT)  (ACT, psum->sbuf, f32r out; no max-subtraction needed:
                        |scores/32| <= ~1.5 for N(0,1) inputs)
  outT_aug[65,q] = sum_k V_aug[k,65].T @ expT[k,q]  (V augmented with a ones
                        column -> row 64 = softmax denominator)
  normalize via reciprocal + PE outer-product broadcast, then
  final[q,n] = outT_norm.T @ W_H.T + b_H  (contract over 1024 = 8 chunks)
"""
import sys
import os

sys.path.insert(0, "/opt/trn_rl_repo")

import numpy as np
import ml_dtypes

B, L, D, H, HD = 4, 1024, 1024, 16, 64
NCORES = 8
QBLK = L // 2  # 512 queries per core
SCALE = 1.0 / np.sqrt(np.float32(D))

_STATE = {}


def _build_nc(niter=1):
    import concourse.bass as bass
    import concourse.tile as tile
    from concourse import bacc, mybir
    from contextlib import ExitStack

    F32 = mybir.dt.float32
    F32R = mybir.dt.float32r
    BF16 = mybir.dt.bfloat16
    Exp = mybir.ActivationFunctionType.Exp

    nc = bacc.Bacc("TRN2", target_bir_lowering=False, debug=False)
    qt = nc.dram_tensor("qt", [128, 8, QBLK], F32R, kind="ExternalInput")
    kt = nc.dram_tensor("kt", [128, 8, L], F32R, kind="ExternalInput")
    vv = nc.dram_tensor("vv", [128, H, 8, HD + 1], F32R, kind="ExternalInput")
    wht = nc.dram_tensor("wht", [128, 8, D], F32R, kind="ExternalInput")
    bias = nc.dram_tensor("bias", [128, D], F32, kind="ExternalInput")
    out = nc.dram_tensor("out", [QBLK, D], F32, kind="ExternalOutput")

    with tile.TileContext(nc) as tc, ExitStack() as ctx:
        singles = ctx.enter_context(tc.tile_pool(name="singles", bufs=1))
        exp_pool = ctx.enter_context(tc.tile_pool(name="exp", bufs=2))
        norm_pool = ctx.enter_context(tc.tile_pool(name="norm", bufs=3))
        final_pool = ctx.enter_context(tc.tile_pool(name="final", bufs=2))
        scores_ps = ctx.enter_context(tc.tile_pool(name="scps", bufs=2, space="PSUM"))
        ov_ps = ctx.enter_context(tc.tile_pool(name="ovps", bufs=1, space="PSUM"))
        bc_ps = ctx.enter_context(tc.tile_pool(name="bcps", bufs=1, space="PSUM"))
        proj_ps = ctx.enter_context(tc.tile_pool(name="prps", bufs=2, space="PSUM"))

        def body(_=None):
            # warm the exp table while DMAs run
            warm_in = singles.tile([1, 8], F32, tag="warm_in")
            warm_out = singles.tile([1, 8], F32, tag="warm_out")
            nc.vector.memset(warm_in, 0.0)
            nc.scalar.activation(out=warm_out, in_=warm_in, func=Exp)

            sb_ones_f = singles.tile([1, HD], F32, tag="ones_f")
            nc.vector.memset(sb_ones_f, 1.0)
            sb_ones = singles.tile([1, HD], F32R, tag="ones")
            nc.vector.tensor_copy(out=sb_ones, in_=sb_ones_f)
            sb_bias = singles.tile([128, D], F32, tag="bias")
            nc.sync.dma_start(sb_bias, bias.ap())

            sb_qt = singles.tile([128, 8, QBLK], F32R, tag="qt")
            sb_kt = singles.tile([128, 8, L], F32R, tag="kt")
            sb_v = singles.tile([128, H, 8, HD + 1], F32R, tag="v")
            for j in range(8):
                nc.sync.dma_start(sb_qt[:, j], qt.ap()[:, j])
                nc.sync.dma_start(sb_kt[:, j], kt.ap()[:, j])
                nc.sync.dma_start(sb_v[:, 2 * j], vv.ap()[:, 2 * j])
                nc.sync.dma_start(sb_v[:, 2 * j + 1], vv.ap()[:, 2 * j + 1])
            sb_wht = singles.tile([128, 8, D], F32R, tag="wht")
            for cc in range(8):
                nc.sync.dma_start(sb_wht[:, cc], wht.ap()[:, cc])

            # normalized concatenated attention output, transposed: [hd, q]
            outT = singles.tile([128, 8, QBLK], F32R, tag="outT")

            for h in range(H):
                j, par = divmod(h, 2)
                poff = par * HD
                expT = exp_pool.tile([128, 8, QBLK], F32R, tag="expT")
                ov = ov_ps.tile([HD + 1, QBLK], F32, tag="ov")

                def scores_subround(r):
                    S = scores_ps.tile([128, 2, QBLK], F32, tag="S")
                    for i in range(2):
                        c = 2 * r + i
                        nc.tensor.matmul(
                            S[:, i, :],
                            lhsT=sb_kt[poff:poff + HD, j, c * 128:(c + 1) * 128],
                            rhs=sb_qt[poff:poff + HD, j, :],
                            start=True,
                            stop=True,
                        )
                    nc.scalar.activation(
                        out=expT[:, 2 * r:2 * r + 2, :], in_=S[:, :, :], func=Exp
                    )

                def attnv(r):
                    for i in range(2):
                        c = 2 * r + i
                        nc.tensor.matmul(
                            ov[:, :],
                            lhsT=sb_v[:, h, c, :],
                            rhs=expT[:, c, :],
                            start=(c == 0),
                            stop=(c == 7),
                        )

                scores_subround(0)
                scores_subround(1)
                attnv(0)
                scores_subround(2)
                attnv(1)
                scores_subround(3)
                attnv(2)
                attnv(3)

                recip = norm_pool.tile([1, QBLK], F32R, tag="recip")
                with nc.allow_low_precision(reason="f32r recip feeds f32r matmul"):
                    nc.vector.reciprocal(out=recip, in_=ov[HD:HD + 1, :])
                bc = bc_ps.tile([HD, QBLK], F32, tag="bc")
                # fp32 outer product: ones[1,64].T @ recip[1,512] -> [64,512]
                nc.tensor.matmul(bc, lhsT=sb_ones, rhs=recip, start=True, stop=True)
                bc_sb = norm_pool.tile([HD, QBLK], F32, tag="bc_sb")
                nc.vector.tensor_copy(out=bc_sb, in_=bc)
                if par == 0:
                    nc.vector.tensor_mul(
                        out=outT[0:HD, j, :], in0=ov[0:HD, :], in1=bc_sb
                    )
                else:
                    tmp = norm_pool.tile([HD, QBLK], F32R, tag="tmp")
                    nc.vector.tensor_mul(out=tmp, in0=ov[0:HD, :], in1=bc_sb)
                    nc.sync.dma_start(outT[HD:128, j, :], tmp)

            # output projection: final[q, n] = outT.T @ WHT + bias
            for m in range(QBLK // 128):
                for jn in range(D // 512):
                    P = proj_ps.tile([128, 512], F32, tag="P")
                    for cc in range(8):
                        nc.tensor.matmul(
                            P,
                            lhsT=outT[:, cc, m * 128:(m + 1) * 128],
                            rhs=sb_wht[:, cc, jn * 512:(jn + 1) * 512],
                            start=(cc == 0),
                            stop=(cc == 7),
                        )
                    F = final_pool.tile([128, 512], F32, tag="F")
                    nc.vector.tensor_add(
                        out=F, in0=P, in1=sb_bias[:, jn * 512:(jn + 1) * 512]
                    )
                    nc.sync.dma_start(
                        out.ap()[m * 128:(m + 1) * 128, jn * 512:(jn + 1) * 512], F
                    )

        if niter == 1:
            body()
        else:
            with tc.For_i(
                0, niter, 1,
                hint_engines=(
                    mybir.EngineType.PE,
                    mybir.EngineType.Activation,
                    mybir.EngineType.DVE,
                    mybir.EngineType.SP,
                ),
            ) as _i:
                body(_i)

    nc.compile()
    return nc


def _host_shard(Q, K, V, W_H, b_H):
    """Build the 8 per-core input dicts (all host-side numpy)."""
    Qs = (np.asarray(Q, np.float32) * SCALE)
    K = np.asarray(K, np.float32)
    V = np.asarray(V, np.float32)
    W_H = np.asarray(W_H, np.float32)
    b_H = np.asarray(b_H, np.float32)

    # [hd, n] chunked: [128, 8, D]
    wht = np.ascontiguousarray(
        W_H.T.reshape(8, 128, D).transpose(1, 0, 2)
    )
    bias = np.ascontiguousarray(np.broadcast_to(b_H, (128, D))).astype(np.float32)

    in_maps = []
    for c in range(NCORES):
        b, half = divmod(c, 2)
        qlo = half * QBLK
        # [q, j, par, d] -> [par, d, j, q] -> [128, 8, QBLK]
        qt = np.ascontiguousarray(
            Qs[b, qlo:qlo + QBLK].reshape(QBLK, 8, 2, HD).transpose(2, 3, 1, 0)
        ).reshape(128, 8, QBLK)
        kt = np.ascontiguousarray(
            K[b].reshape(L, 8, 2, HD).transpose(2, 3, 1, 0)
        ).reshape(128, 8, L)
        # V_aug [k, h, 65] -> [c, p, h, e] -> [p, h, c, e]
        va = np.concatenate(
            [V[b].reshape(L, H, HD), np.ones((L, H, 1), np.float32)], axis=2
        )
        vv = np.ascontiguousarray(
            va.reshape(8, 128, H, HD + 1).transpose(1, 2, 0, 3)
        )
        in_maps.append({"qt": qt, "kt": kt, "vv": vv, "wht": wht, "bias": bias})
    return in_maps


def _get_runner(niter=1):
    """Build (once) and cache a jitted 8-core runner for the kernel."""
    key = ("runner", niter)
    if key in _STATE:
        return _STATE[key]

    import jax
    import jax.numpy as jnp
    from jax.sharding import Mesh, PartitionSpec
    from jax.experimental.shard_map import shard_map
    from concourse import bass2jax, mybir

    nc = _build_nc(niter)
    bass2jax.install_neuronx_cc_hook()

    partition_name = (
        nc.partition_id_tensor.name if nc.partition_id_tensor else None
    )
    in_names, out_names, out_avals, zero_shapes = [], [], [], []
    for alloc in nc.m.functions[0].allocations:
        if not isinstance(alloc, mybir.MemoryLocationSet):
            continue
        name = alloc.memorylocations[0].name
        if alloc.kind == "ExternalInput":
            if name != partition_name:
                in_names.append(name)
        elif alloc.kind == "ExternalOutput":
            out_names.append(name)
            shape = tuple(alloc.tensor_shape)
            dtype = mybir.dt.np(alloc.dtype)
            out_avals.append(jax.core.ShapedArray(shape, dtype))
            zero_shapes.append((shape, dtype))
    n_params = len(in_names)
    n_outs = len(out_avals)
    all_names = list(in_names) + list(out_names)
    if partition_name is not None:
        all_names.append(partition_name)
    donate = tuple(range(n_params, n_params + n_outs))

    def _body(*args):
        operands = list(args)
        if partition_name is not None:
            operands.append(bass2jax.partition_id_tensor())
        outs = bass2jax._bass_exec_p.bind(
            *operands,
            out_avals=tuple(out_avals),
            in_names=tuple(all_names),
            out_names=tuple(out_names),
            lowering_input_output_aliases=(),
            sim_require_finite=True,
            sim_require_nnan=True,
            nc=nc,
        )
        return tuple(outs)

    devices = jax.devices()[:NCORES]
    mesh = Mesh(np.asarray(devices), ("core",))
    in_specs = (PartitionSpec("core"),) * (n_params + n_outs)
    out_specs = (PartitionSpec("core"),) * n_outs
    sharded = jax.jit(
        shard_map(
            _body, mesh=mesh, in_specs=in_specs, out_specs=out_specs,
            check_rep=False,
        ),
        donate_argnums=donate,
        keep_unused=True,
    )

    def run(in_maps, device_inputs=None):
        if device_inputs is None:
            device_inputs = put_inputs(in_maps)
        sharding = jax.sharding.NamedSharding(mesh, PartitionSpec("core"))
        zeros = [
            jax.device_put(np.zeros((NCORES * s[0], *s[1:]), d), sharding)
            for s, d in zero_shapes
        ]
        out_arrs = sharded(*device_inputs, *zeros)
        results = []
        for c in range(NCORES):
            results.append({
                name: np.asarray(out_arrs[i]).reshape(
                    NCORES, *out_avals[i].shape
                )[c]
                for i, name in enumerate(out_names)
            })
        return results

    def put_inputs(in_maps):
        sharding = jax.sharding.NamedSharding(mesh, PartitionSpec("core"))
        return [
            jax.device_put(
                np.concatenate(
                    [np.asarray(in_maps[c][nm]) for c in range(NCORES)], axis=0
                ),
                sharding,
            )
            for nm in in_names
        ]

    runner = {"run": run, "put_inputs": put_inputs, "sharded": sharded,
              "in_names": in_names, "out_names": out_names,
              "zero_shapes": zero_shapes, "nc": nc}
    _STATE[key] = runner
    return runner


def kernel(Q=None, K=None, V=None, W_H=None, b_H=None, mask=None, **kw):
    in_maps = _host_shard(Q, K, V, W_H, b_H)
    runner = _get_runner(niter=1)
    results = runner["run"](in_maps)
    out = np.empty((B, L, D), np.float32)
    for c in range(NCORES):
        b, half = divmod(c, 2)
        out[b, half * QBLK:(half + 1) * QBLK, :] = results[c]["out"]
    return out
